# revision 12
# baseline (speedup 1.0000x reference)
"""MoE (T=2048 H=2048 I=1408 E=16 top-2) on 8 trn2 NeuronCores.

Strategy (expert-parallel, per the sharding hint):
  - Router (gate linear + top-2 sigmoid + renorm) computed on host in f64
    (matches the f32 reference's top-k selections with margin to spare).
  - Tokens are dispatched host-side: experts sorted by load; the 8 heaviest
    go in slot 0 (capacity C0 = max load) and the 8 lightest in slot 1
    (capacity C1), one of each per core.
  - Each core runs a Bass/Tile kernel computing, per owned expert:
        gT = Wg_e.T @ X_e.T   uT = Wu_e.T @ X_e.T        (bf16 matmul)
        hT = silu(gT) * uT                                (fp32, cast bf16)
        yT = Wd_e.T(row-tiles) @ hT                       (bf16 out, unscaled)
    fp32 accumulation stays in PSUM.  Outputs are bf16 and unscaled; the
    per-token combine weights are applied host-side during the gather.

  Scheduling (the kernel is at the compute/memory ridge: PE ~119us of
  bf16 matmul vs ~109us of DMA at the ~360GB/s pipe):
  - One HW-DGE ring (sync engine, q1) carries the whole weight stream in
    exact consumption order: xt0 chunks, e0's first two i-tiles split
    per-plane ([P,H] halves so the i0/i1 races clear), fused [P,2,H]
    slabs for i>=2, e0's down slabs, then e1's stream.  The scalar ring
    (q10) carries only xt1 (issued during e0's down phase) and outputs.
  - Down projection runs in h-quarters of 4 PSUM banks (5-deep py pool):
    quarter 0 iterates i-outer so the 11 wd slabs are first-needed spread
    over ~5us (the DMA stream lands them just in time); quarters 1-3
    iterate h-outer so each PSUM group closes early and its bank recycles
    into the next quarter without stalling the PE.
  - PSUM: gate/up share one 3-bank ring (pg/pu alternate), down uses 5.
  - ~30 warm-up matmuls bridge the DGE/DMA latency so the PE clock has
    ramped and never idles (idle gaps reset the clock to half speed for
    ~3us) before the first real matmul's inputs land.
  - Host combines: out[t] = w0*Y[:,col0(t)] + w1*Y[:,col1(t)].
"""

import numpy as np
import ml_dtypes

import concourse.bacc as bacc
import concourse.mybir as mybir
import concourse.tile as tile
from concourse.bass_utils import run_bass_kernel_spmd

T = 2048
H = 2048
I = 1408
E = 16
K = 2
NCORES = 8
EPC = E // NCORES  # experts per core (2)
P = 128
HT = H // P        # 16 h-tiles
IT = I // P        # 11 i-tiles

BF16 = mybir.dt.bfloat16
F32 = mybir.dt.float32
nbf = ml_dtypes.bfloat16

_kernel_cache: dict[tuple, tuple] = {}
_weight_cache: dict[tuple, tuple] = {}
import os as _os
_WARMUP = int(_os.environ.get("K_WARMUP", "56"))

# xt chunk spans (h-tile ranges) per slot
XT_CHUNKS = ([(0, 6), (6, 11), (11, 16)], [(0, 8), (8, 16)])
# down-phase h-quarters (4 PSUM banks each)
QUARTERS = [(0, 4), (4, 8), (8, 12), (12, 16)]


def _build(caps: tuple[int, ...]):
    """Build + compile the per-core kernel for slot capacities `caps`."""
    Cm = max(caps)

    nc = bacc.Bacc("TRN2", target_bir_lowering=False, debug=False, num_devices=NCORES)
    # activations, token-gathered+transposed per slot:
    #   xt{j}[p, h, k] = X[token_k_of_slot_j, h*P + p]
    xts_d = [
        nc.dram_tensor(f"xt{j}", [P, HT, caps[j]], BF16, kind="ExternalInput")
        for j in range(EPC)
    ]
    # fused gate+up weight slabs (4KB rows):
    #   wgu[e, i, 0, p, h*P + c] = Wg[e, h*P + p, i*P + c]
    #   wgu[e, i, 1, p, h*P + c] = Wu[e, h*P + p, i*P + c]
    wgu = nc.dram_tensor("wgu", [EPC, IT, 2, P, H], BF16, kind="ExternalInput")
    #   wds[e, i, p, :] = Wd[e, i*P + p, :]
    wds = nc.dram_tensor("wds", [EPC, IT, P, H], BF16, kind="ExternalInput")
    # unscaled bf16 expert outputs:  yo{j}[p, h, k] = y_j[h*P + p, token_k]
    yos_d = [
        nc.dram_tensor(f"yo{j}", [P, HT, caps[j]], BF16, kind="ExternalOutput")
        for j in range(EPC)
    ]

    with tile.TileContext(nc) as tc:
        with (
            tc.tile_pool(name="xt_pool", bufs=3) as xt_pool,
            tc.tile_pool(name="wgu_pool", bufs=6) as wgu_pool,
            tc.tile_pool(name="wd_pool", bufs=IT) as wd_pool,
            tc.tile_pool(name="ht_pool", bufs=IT) as ht_pool,
            tc.tile_pool(name="tmp_pool", bufs=2) as tmp_pool,
            tc.tile_pool(name="out_pool", bufs=2) as out_pool,
            tc.tile_pool(name="pgu_pool", bufs=3, space="PSUM") as pgu_pool,
            tc.tile_pool(name="py_pool", bufs=5, space="PSUM") as py_pool,
        ):
            # --- startup: the two HW-DGE rings start in parallel — the
            # scalar ring (q10) carries xt0's chunks while the sync ring
            # (q1) starts the weight stream.  xt1's tiles share the xt
            # pool's slots, which naturally delays their transfers until
            # xt0 dies at the end of e0's gate/up (keeping the startup
            # window's bandwidth for the critical stream).
            xt_tiles: list[list] = [[], []]

            def _xt_chunk(j, ci, eng):
                h0, h1 = XT_CHUNKS[j][ci]
                t_ = xt_pool.tile(
                    [P, h1 - h0, caps[j]], BF16, tag="xt", name=f"xt{j}c{ci}"
                )
                eng.dma_start(t_[:], xts_d[j].ap()[:, h0:h1, :])
                xt_tiles[j].append((h0, h1, t_))

            def _issue_xt(j, eng):
                for ci in range(len(XT_CHUNKS[j])):
                    _xt_chunk(j, ci, eng)

            def xt_slice(j, h):
                for h0, h1, t_ in xt_tiles[j]:
                    if h0 <= h < h1:
                        return t_[:, h - h0, :]
                raise AssertionError

            # e0 i0/i1 weights as separate [P, H] planes (finer arrival
            # granularity for the startup race; source rows stay 4KB).
            # They share the wd pool's [P, H] slots — all four are dead by
            # the time the pool wraps around to reuse them.  Everything the
            # first i-tile needs rides the sync ring (the scalar ring's
            # first transfer starts ~2.5us later), interleaved in first-use
            # order; i1's planes ride the scalar ring.
            w0_tiles = {}

            def _w0(i, g, eng):
                t_ = wd_pool.tile([P, H], BF16, tag="wd", name=f"w0_{i}_{g}")
                eng.dma_start(t_[:], wgu.ap()[0, i, g])
                w0_tiles[(i, g)] = t_

            _xt_chunk(0, 0, nc.sync)
            _w0(0, 0, nc.sync)
            _xt_chunk(0, 1, nc.sync)
            _xt_chunk(0, 2, nc.sync)
            _w0(0, 1, nc.sync)
            _w0(1, 0, nc.scalar)
            _w0(1, 1, nc.scalar)

            # --- PE warm-up: throwaway matmuls while the startup DMAs land,
            # keeping the PE busy (and its clock ramping) until real work.
            if _WARMUP:
                warm_sb = tmp_pool.tile([P, P], BF16, tag="tmp", name="warm")
                nc.gpsimd.memset(warm_sb[:], 0)
                for _ in range(_WARMUP):
                    pw = py_pool.tile([P, P], F32, tag="py")
                    nc.tensor.matmul(
                        pw[:], warm_sb[:], warm_sb[:], start=True, stop=True
                    )

            wd_map = [[None] * IT for _ in range(EPC)]

            def _wd(ee, i, eng):
                t_ = wd_pool.tile([P, H], BF16, tag="wd", name=f"wd{ee}_{i}")
                eng.dma_start(t_[:], wds.ap()[ee, i])
                wd_map[ee][i] = t_

            for e in range(EPC):
                C = caps[e]
                # ---- gate/up projections ----
                # e0's down slabs ride the sync ring, interleaved 1:1 with
                # the gate slabs from i=3 on so they land just behind them
                # (and the sync sequencer never holds many issues hostage
                # behind one pool-gated issue).  e1's down slabs ride the
                # scalar ring (idle then), so they never contend with e1's
                # gate stream for FIFO order.
                hts = []
                for i in range(IT):
                    if e == 0 and i < 2:
                        g_sl = lambda h, t_=w0_tiles[(i, 0)]: t_[:, h * P:(h + 1) * P]
                        u_sl = lambda h, t_=w0_tiles[(i, 1)]: t_[:, h * P:(h + 1) * P]
                    else:
                        wgu_t = wgu_pool.tile([P, 2, H], BF16, tag="wgu")
                        nc.sync.dma_start(
                            wgu_t[:], wgu.ap()[e, i].rearrange("g p h -> p g h")
                        )
                        g_sl = lambda h, t_=wgu_t: t_[:, 0, h * P:(h + 1) * P]
                        u_sl = lambda h, t_=wgu_t: t_[:, 1, h * P:(h + 1) * P]
                    if e == 0 and i >= 3:
                        _wd(0, i - 3, nc.sync)
                    if e == 1:
                        _wd(1, i, nc.scalar)
                    pg = pgu_pool.tile([P, Cm], F32, tag="pgu", name=f"pg{e}_{i}")
                    pu = pgu_pool.tile([P, Cm], F32, tag="pgu", name=f"pu{e}_{i}")
                    for h in range(HT):
                        nc.tensor.matmul(
                            pg[:, :C], g_sl(h), xt_slice(e, h),
                            start=(h == 0), stop=(h == HT - 1),
                        )
                    for h in range(HT):
                        nc.tensor.matmul(
                            pu[:, :C], u_sl(h), xt_slice(e, h),
                            start=(h == 0), stop=(h == HT - 1),
                        )
                    tmp = tmp_pool.tile([P, Cm], F32, tag="tmp")
                    nc.scalar.activation(
                        tmp[:, :C], pg[:, :C], mybir.ActivationFunctionType.Silu
                    )
                    ht_t = ht_pool.tile([P, Cm], BF16, tag="ht")
                    nc.vector.tensor_tensor(
                        ht_t[:, :C], tmp[:, :C], pu[:, :C], mybir.AluOpType.mult
                    )
                    hts.append(ht_t)

                if e == 0:
                    for i in range(IT - 3, IT):
                        _wd(0, i, nc.sync)
                    # slot-1 activations: tiles reuse xt0's slots, so their
                    # transfers start once xt0 dies at e0 gate/up end.
                    _issue_xt(1, nc.scalar)
                wd_tiles = wd_map[e]

                # ---- down projection (transposed bf16 output) ----
                # h-quarters of 4 PSUM banks.  Quarter 0 runs i-outer so wd
                # slabs are first-needed spread in time; quarters 1-3 run
                # h-outer so each group closes early and its bank recycles.
                last = e == EPC - 1
                for qi, (h0, h1) in enumerate(QUARTERS):
                    pys = [
                        py_pool.tile([P, Cm], F32, tag="py", name=f"py{e}_{h}")
                        for h in range(h0, h1)
                    ]
                    if qi == 0:
                        for i in range(IT):
                            for hh, h in enumerate(range(h0, h1)):
                                nc.tensor.matmul(
                                    pys[hh][:, :C],
                                    wd_tiles[i][:, h * P:(h + 1) * P],
                                    hts[i][:, :C],
                                    start=(i == 0), stop=(i == IT - 1),
                                )
                    else:
                        for hh, h in enumerate(range(h0, h1)):
                            for i in range(IT):
                                nc.tensor.matmul(
                                    pys[hh][:, :C],
                                    wd_tiles[i][:, h * P:(h + 1) * P],
                                    hts[i][:, :C],
                                    start=(i == 0), stop=(i == IT - 1),
                                )
                    if last and qi == len(QUARTERS) - 1:
                        # final quarter: per-h outputs so only ~0.13MB of
                        # DMA trails the last matmul.
                        for hh, h in enumerate(range(h0, h1)):
                            oc = out_pool.tile(
                                [P, 1, C], BF16, tag="oc1", bufs=2, name=f"oc1_{h}"
                            )
                            nc.vector.tensor_scalar_mul(
                                oc[:, 0, :], pys[hh][:, :C], 1.0
                            )
                            nc.scalar.dma_start(
                                yos_d[e].ap()[:, h:h + 1, :], oc[:]
                            )
                    else:
                        oc = out_pool.tile(
                            [P, h1 - h0, C], BF16, tag="oc", name=f"oc{e}_{qi}"
                        )
                        for hh, h in enumerate(range(h0, h1)):
                            nc.vector.tensor_scalar_mul(
                                oc[:, hh, :], pys[hh][:, :C], 1.0
                            )
                        nc.scalar.dma_start(yos_d[e].ap()[:, h0:h1, :], oc[:])

    nc.compile()
    return nc


def _get_kernel(caps):
    if caps not in _kernel_cache:
        _kernel_cache[caps] = _build(caps)
    return _kernel_cache[caps]


def _prep_weights(w_gate_proj, w_up_proj, w_down_proj):
    key = tuple(
        (a.__array_interface__["data"][0], a.shape)
        for a in (w_gate_proj, w_up_proj, w_down_proj)
    )
    if key in _weight_cache:
        return _weight_cache[key]
    wg_bf = np.asarray(w_gate_proj, np.float32).astype(nbf)  # [E, H, I]
    wu_bf = np.asarray(w_up_proj, np.float32).astype(nbf)    # [E, H, I]
    wd_bf = np.asarray(w_down_proj, np.float32).astype(nbf)  # [E, I, H]
    wg_slab = wg_bf.reshape(E, HT, P, IT, P).transpose(0, 3, 2, 1, 4).reshape(E, IT, P, H)
    wu_slab = wu_bf.reshape(E, HT, P, IT, P).transpose(0, 3, 2, 1, 4).reshape(E, IT, P, H)
    wgu_all = np.ascontiguousarray(np.stack([wg_slab, wu_slab], axis=2))  # [E, IT, 2, P, H]
    wd_rows = np.ascontiguousarray(wd_bf.reshape(E, IT, P, H))
    _weight_cache.clear()
    _weight_cache[key] = (wgu_all, wd_rows)
    return _weight_cache[key]


def _route(X, WG):
    """f64 replica of the reference router; returns per-expert dispatch."""
    logits = X.astype(np.float64) @ np.asarray(WG, np.float64)
    scores = 1.0 / (1.0 + np.exp(-logits))
    top2 = np.argsort(-scores, axis=1, kind="stable")[:, :K]
    w = np.take_along_axis(scores, top2, 1)
    wn = (w / w.sum(1, keepdims=True)).astype(np.float32)
    tok_list, w_list = [], []
    for e in range(E):
        hit = top2 == e  # [T, K]
        tok = np.nonzero(hit.any(1))[0]
        kk = hit[tok, 1].astype(np.int64)
        tok_list.append(tok)
        w_list.append(wn[tok, kk])
    return tok_list, w_list


def _pack_xt(X, tokens, cap):
    """[P, HT, cap] bf16: xt[p, h, k] = X[tokens[k], h*P + p]."""
    n = len(tokens)
    Xg = np.zeros((cap, H), np.float32)
    Xg[:n] = X[tokens]
    arr = Xg.T.reshape(HT, P, cap).transpose(1, 0, 2)
    return np.ascontiguousarray(arr.astype(nbf))


def _run(inputs: dict, trace: bool = False, trace_cores=None):
    X = np.ascontiguousarray(np.asarray(inputs["hidden_states"], np.float32))
    tok_list, w_list = _route(X, inputs["w_gate"])
    counts = np.array([len(t) for t in tok_list])

    # slot assignment: lightest 8 experts in slot 0 (smaller startup mass
    # eases the initial DMA race), heaviest 8 in slot 1
    order = np.argsort(-counts, kind="stable")
    slot_exp = [order[NCORES:], order[:NCORES]]  # [slot][core] -> expert
    caps = tuple(
        min(512, max(16, int(counts[slot_exp[j]].max())))
        for j in range(EPC)
    )
    if counts.max() > 512:
        raise RuntimeError(f"expert load {counts.max()} exceeds supported capacity")
    nc = _get_kernel(caps)
    wgu_all, wd_rows = _prep_weights(
        inputs["w_gate_proj"], inputs["w_up_proj"], inputs["w_down_proj"]
    )

    in_maps = []
    for c in range(NCORES):
        experts = [int(slot_exp[j][c]) for j in range(EPC)]
        m = {
            "wgu": np.ascontiguousarray(wgu_all[experts]),
            "wds": np.ascontiguousarray(wd_rows[experts]),
        }
        for j, e in enumerate(experts):
            m[f"xt{j}"] = _pack_xt(X, tok_list[e], caps[j])
        in_maps.append(m)

    if trace:
        _install_trace_shim()
    res = run_bass_kernel_spmd(
        nc,
        in_maps,
        core_ids=list(range(NCORES)),
        trace=trace,
        trace_cores=trace_cores,
    )

    # combine on host: out[t] = w0*Y[:, col0] + w1*Y[:, col1]
    # big: [H, NCORES*(C0+C1)] in (core, slot) column order
    col_blocks = []
    for c in range(NCORES):
        for j in range(EPC):
            y = np.asarray(res.results[c][f"yo{j}"], np.float32)  # [P, HT, Cj]
            col_blocks.append(y.transpose(1, 0, 2).reshape(H, caps[j]))
    big = np.concatenate(col_blocks, axis=1)
    TCc = sum(caps)

    col_a = np.full(T, -1, np.int64)
    col_b = np.full(T, -1, np.int64)
    w_a = np.zeros(T, np.float32)
    w_b = np.zeros(T, np.float32)
    for j in range(EPC):
        base_j = sum(caps[:j])
        for c in range(NCORES):
            e = int(slot_exp[j][c])
            tok = tok_list[e]
            cols = c * TCc + base_j + np.arange(counts[e])
            first = col_a[tok] < 0
            col_a[tok[first]] = cols[first]
            w_a[tok[first]] = w_list[e][first]
            col_b[tok[~first]] = cols[~first]
            w_b[tok[~first]] = w_list[e][~first]
    assert (col_a >= 0).all() and (col_b >= 0).all()
    out = (big[:, col_a] * w_a[None, :] + big[:, col_b] * w_b[None, :]).T
    return np.ascontiguousarray(out.astype(np.float32)), res


def kernel(**inputs) -> np.ndarray:
    out, _ = _run(inputs, trace=False)
    return out


def _install_trace_shim():
    """Make run_bass_kernel_spmd(trace=True) work under axon: register the
    NTFF profile hook that the slim agent image's antenv stub lacks."""
    import sys, types

    if "antenv.axon_hooks" not in sys.modules:
        import antenv

        mod = types.ModuleType("antenv.axon_hooks")
        mod._hook = None
        mod.set_axon_ntff_profile_hook = lambda h: setattr(mod, "_hook", h)
        mod.get_axon_ntff_profile_hook = lambda: mod._hook
        sys.modules["antenv.axon_hooks"] = mod
        antenv.axon_hooks = mod
    if sys.modules["antenv.axon_hooks"].get_axon_ntff_profile_hook() is None:
        from trn_agent_boot.trn_boot import _ntff_profile_via_ctypes

        sys.modules["antenv.axon_hooks"].set_axon_ntff_profile_hook(
            _ntff_profile_via_ctypes("/opt/axon/libaxon_pjrt.so")
        )


# revision 13
# speedup vs baseline: 1.0286x; 1.0286x over previous
"""MoE (T=2048 H=2048 I=1408 E=16 top-2) on 8 trn2 NeuronCores.

Strategy (expert-parallel, per the sharding hint):
  - Router (gate linear + top-2 sigmoid + renorm) computed on host in f64
    (matches the f32 reference's top-k selections with margin to spare).
  - Tokens are dispatched host-side: experts sorted by load; the 8 heaviest
    go in slot 0 (capacity C0 = max load) and the 8 lightest in slot 1
    (capacity C1), one of each per core.
  - Each core runs a Bass/Tile kernel computing, per owned expert:
        gT = Wg_e.T @ X_e.T   uT = Wu_e.T @ X_e.T        (bf16 matmul)
        hT = silu(gT) * uT                                (fp32, cast bf16)
        yT = Wd_e.T(row-tiles) @ hT                       (bf16 out, unscaled)
    fp32 accumulation stays in PSUM.  Outputs are bf16 and unscaled; the
    per-token combine weights are applied host-side during the gather.

  Scheduling (the kernel is at the compute/memory ridge: PE ~119us of
  bf16 matmul vs ~109us of DMA at the ~360GB/s pipe):
  - One HW-DGE ring (sync engine, q1) carries the whole weight stream in
    exact consumption order: xt0 chunks, e0's first two i-tiles split
    per-plane ([P,H] halves so the i0/i1 races clear), fused [P,2,H]
    slabs for i>=2, e0's down slabs, then e1's stream.  The scalar ring
    (q10) carries only xt1 (issued during e0's down phase) and outputs.
  - Down projection runs in h-quarters of 4 PSUM banks (5-deep py pool):
    quarter 0 iterates i-outer so the 11 wd slabs are first-needed spread
    over ~5us (the DMA stream lands them just in time); quarters 1-3
    iterate h-outer so each PSUM group closes early and its bank recycles
    into the next quarter without stalling the PE.
  - PSUM: gate/up share one 3-bank ring (pg/pu alternate), down uses 5.
  - ~30 warm-up matmuls bridge the DGE/DMA latency so the PE clock has
    ramped and never idles (idle gaps reset the clock to half speed for
    ~3us) before the first real matmul's inputs land.
  - Host combines: out[t] = w0*Y[:,col0(t)] + w1*Y[:,col1(t)].
"""

import numpy as np
import ml_dtypes

import concourse.bacc as bacc
import concourse.mybir as mybir
import concourse.tile as tile
from concourse.bass_utils import run_bass_kernel_spmd

T = 2048
H = 2048
I = 1408
E = 16
K = 2
NCORES = 8
EPC = E // NCORES  # experts per core (2)
P = 128
HT = H // P        # 16 h-tiles
IT = I // P        # 11 i-tiles

BF16 = mybir.dt.bfloat16
F32 = mybir.dt.float32
nbf = ml_dtypes.bfloat16

_kernel_cache: dict[tuple, tuple] = {}
_weight_cache: dict[tuple, tuple] = {}
import os as _os
_WARMUP = int(_os.environ.get("K_WARMUP", "74"))

# xt chunk spans (h-tile ranges) per slot: one transfer each — per-transfer
# queue overhead (~2us) dwarfs chunking gains, and xt1's single tile shares
# xt0's pool slot so its transfer is naturally gated behind xt0's death
XT_CHUNKS = ([(0, 16)], [(0, 16)])
# down-phase h-quarters (4 PSUM banks each)
QUARTERS = [(0, 4), (4, 8), (8, 12), (12, 16)]


def _build(caps: tuple[int, ...]):
    """Build + compile the per-core kernel for slot capacities `caps`."""
    Cm = max(caps)

    nc = bacc.Bacc("TRN2", target_bir_lowering=False, debug=False, num_devices=NCORES)
    # activations, token-gathered+transposed per slot:
    #   xt{j}[p, h, k] = X[token_k_of_slot_j, h*P + p]
    xts_d = [
        nc.dram_tensor(f"xt{j}", [P, HT, caps[j]], BF16, kind="ExternalInput")
        for j in range(EPC)
    ]
    # fused gate+up weight slabs (4KB rows):
    #   wgu[e, i, 0, p, h*P + c] = Wg[e, h*P + p, i*P + c]
    #   wgu[e, i, 1, p, h*P + c] = Wu[e, h*P + p, i*P + c]
    wgu = nc.dram_tensor("wgu", [EPC, IT, 2, P, H], BF16, kind="ExternalInput")
    #   wds[e, i, p, :] = Wd[e, i*P + p, :]
    wds = nc.dram_tensor("wds", [EPC, IT, P, H], BF16, kind="ExternalInput")
    # unscaled bf16 expert outputs:  yo{j}[p, h, k] = y_j[h*P + p, token_k]
    yos_d = [
        nc.dram_tensor(f"yo{j}", [P, HT, caps[j]], BF16, kind="ExternalOutput")
        for j in range(EPC)
    ]

    with tile.TileContext(nc) as tc:
        with (
            tc.tile_pool(name="xt_pool", bufs=1) as xt_pool,
            tc.tile_pool(name="wgu_pool", bufs=6) as wgu_pool,
            tc.tile_pool(name="wd_pool", bufs=IT) as wd_pool,
            tc.tile_pool(name="ht_pool", bufs=IT) as ht_pool,
            tc.tile_pool(name="tmp_pool", bufs=2) as tmp_pool,
            tc.tile_pool(name="out_pool", bufs=2) as out_pool,
            tc.tile_pool(name="pgu_pool", bufs=3, space="PSUM") as pgu_pool,
            tc.tile_pool(name="py_pool", bufs=5, space="PSUM") as py_pool,
        ):
            # --- startup: the two HW-DGE rings start in parallel — the
            # scalar ring (q10) carries xt0's chunks while the sync ring
            # (q1) starts the weight stream.  xt1's tiles share the xt
            # pool's slots, which naturally delays their transfers until
            # xt0 dies at the end of e0's gate/up (keeping the startup
            # window's bandwidth for the critical stream).
            xt_tiles: list[list] = [[], []]

            def _xt_chunk(j, ci, eng):
                h0, h1 = XT_CHUNKS[j][ci]
                t_ = xt_pool.tile(
                    [P, h1 - h0, caps[j]], BF16, tag="xt", name=f"xt{j}c{ci}"
                )
                eng.dma_start(t_[:], xts_d[j].ap()[:, h0:h1, :])
                xt_tiles[j].append((h0, h1, t_))

            def _issue_xt(j, eng):
                for ci in range(len(XT_CHUNKS[j])):
                    _xt_chunk(j, ci, eng)

            def xt_slice(j, h):
                for h0, h1, t_ in xt_tiles[j]:
                    if h0 <= h < h1:
                        return t_[:, h - h0, :]
                raise AssertionError

            # startup: xt0 rides the sync ring (first packets ~8us);
            # e0's first fused slab rides the scalar ring in parallel
            # (its first packets start ~2.5us later; the g-plane isn't
            # needed until the first g-matmuls anyway).
            _xt_chunk(0, 0, nc.sync)
            wgu00 = wgu_pool.tile([P, 2, H], BF16, tag="wgu", name="wgu00")
            nc.scalar.dma_start(
                wgu00[:], wgu.ap()[0, 0].rearrange("g p h -> p g h")
            )

            # --- PE warm-up: throwaway matmuls while the startup DMAs land,
            # keeping the PE busy (and its clock ramping) until real work.
            if _WARMUP:
                warm_sb = tmp_pool.tile([P, P], BF16, tag="tmp", name="warm")
                nc.gpsimd.memset(warm_sb[:], 0)
                for _ in range(_WARMUP):
                    pw = py_pool.tile([P, P], F32, tag="py")
                    nc.tensor.matmul(
                        pw[:], warm_sb[:], warm_sb[:], start=True, stop=True
                    )

            wd_map = [[None] * IT for _ in range(EPC)]

            def _wd(ee, i, eng):
                t_ = wd_pool.tile([P, H], BF16, tag="wd", name=f"wd{ee}_{i}")
                eng.dma_start(t_[:], wds.ap()[ee, i])
                wd_map[ee][i] = t_

            for e in range(EPC):
                C = caps[e]
                # ---- gate/up projections ----
                # e0's down slabs ride the sync ring, interleaved 1:1 with
                # the gate slabs from i=3 on so they land just behind them
                # (and the sync sequencer never holds many issues hostage
                # behind one pool-gated issue).  e1's down slabs ride the
                # scalar ring (idle then), so they never contend with e1's
                # gate stream for FIFO order.
                hts = []
                for i in range(IT):
                    if e == 0 and i == 0:
                        g_sl = lambda h, t_=wgu00: t_[:, 0, h * P:(h + 1) * P]
                        u_sl = lambda h, t_=wgu00: t_[:, 1, h * P:(h + 1) * P]
                    else:
                        wgu_t = wgu_pool.tile([P, 2, H], BF16, tag="wgu")
                        nc.sync.dma_start(
                            wgu_t[:], wgu.ap()[e, i].rearrange("g p h -> p g h")
                        )
                        g_sl = lambda h, t_=wgu_t: t_[:, 0, h * P:(h + 1) * P]
                        u_sl = lambda h, t_=wgu_t: t_[:, 1, h * P:(h + 1) * P]
                    if e == 0 and i >= 3:
                        _wd(0, i - 3, nc.sync)
                    if e == 1:
                        _wd(1, i, nc.scalar)
                    pg = pgu_pool.tile([P, Cm], F32, tag="pgu", name=f"pg{e}_{i}")
                    pu = pgu_pool.tile([P, Cm], F32, tag="pgu", name=f"pu{e}_{i}")
                    for h in range(HT):
                        nc.tensor.matmul(
                            pg[:, :C], g_sl(h), xt_slice(e, h),
                            start=(h == 0), stop=(h == HT - 1),
                        )
                    for h in range(HT):
                        nc.tensor.matmul(
                            pu[:, :C], u_sl(h), xt_slice(e, h),
                            start=(h == 0), stop=(h == HT - 1),
                        )
                    tmp = tmp_pool.tile([P, Cm], F32, tag="tmp")
                    nc.scalar.activation(
                        tmp[:, :C], pg[:, :C], mybir.ActivationFunctionType.Silu
                    )
                    ht_t = ht_pool.tile([P, Cm], BF16, tag="ht")
                    nc.vector.tensor_tensor(
                        ht_t[:, :C], tmp[:, :C], pu[:, :C], mybir.AluOpType.mult
                    )
                    hts.append(ht_t)

                if e == 0:
                    for i in range(IT - 3, IT):
                        _wd(0, i, nc.sync)
                    # slot-1 activations: tiles reuse xt0's slots, so their
                    # transfers start once xt0 dies at e0 gate/up end.
                    _issue_xt(1, nc.scalar)
                wd_tiles = wd_map[e]

                # ---- down projection (transposed bf16 output) ----
                # h-quarters of 4 PSUM banks.  Quarter 0 runs i-outer so wd
                # slabs are first-needed spread in time; quarters 1-3 run
                # h-outer so each group closes early and its bank recycles.
                last = e == EPC - 1
                for qi, (h0, h1) in enumerate(QUARTERS):
                    pys = [
                        py_pool.tile([P, Cm], F32, tag="py", name=f"py{e}_{h}")
                        for h in range(h0, h1)
                    ]
                    if qi == 0:
                        for i in range(IT):
                            for hh, h in enumerate(range(h0, h1)):
                                nc.tensor.matmul(
                                    pys[hh][:, :C],
                                    wd_tiles[i][:, h * P:(h + 1) * P],
                                    hts[i][:, :C],
                                    start=(i == 0), stop=(i == IT - 1),
                                )
                    else:
                        for hh, h in enumerate(range(h0, h1)):
                            for i in range(IT):
                                nc.tensor.matmul(
                                    pys[hh][:, :C],
                                    wd_tiles[i][:, h * P:(h + 1) * P],
                                    hts[i][:, :C],
                                    start=(i == 0), stop=(i == IT - 1),
                                )
                    if last and qi == len(QUARTERS) - 1:
                        # final quarter: per-h outputs so only ~0.13MB of
                        # DMA trails the last matmul.
                        for hh, h in enumerate(range(h0, h1)):
                            oc = out_pool.tile(
                                [P, 1, C], BF16, tag="oc1", bufs=2, name=f"oc1_{h}"
                            )
                            nc.vector.tensor_scalar_mul(
                                oc[:, 0, :], pys[hh][:, :C], 1.0
                            )
                            nc.scalar.dma_start(
                                yos_d[e].ap()[:, h:h + 1, :], oc[:]
                            )
                    else:
                        oc = out_pool.tile(
                            [P, h1 - h0, C], BF16, tag="oc", name=f"oc{e}_{qi}"
                        )
                        for hh, h in enumerate(range(h0, h1)):
                            nc.vector.tensor_scalar_mul(
                                oc[:, hh, :], pys[hh][:, :C], 1.0
                            )
                        nc.scalar.dma_start(yos_d[e].ap()[:, h0:h1, :], oc[:])

    nc.compile()
    return nc


def _get_kernel(caps):
    if caps not in _kernel_cache:
        _kernel_cache[caps] = _build(caps)
    return _kernel_cache[caps]


def _prep_weights(w_gate_proj, w_up_proj, w_down_proj):
    key = tuple(
        (a.__array_interface__["data"][0], a.shape)
        for a in (w_gate_proj, w_up_proj, w_down_proj)
    )
    if key in _weight_cache:
        return _weight_cache[key]
    wg_bf = np.asarray(w_gate_proj, np.float32).astype(nbf)  # [E, H, I]
    wu_bf = np.asarray(w_up_proj, np.float32).astype(nbf)    # [E, H, I]
    wd_bf = np.asarray(w_down_proj, np.float32).astype(nbf)  # [E, I, H]
    wg_slab = wg_bf.reshape(E, HT, P, IT, P).transpose(0, 3, 2, 1, 4).reshape(E, IT, P, H)
    wu_slab = wu_bf.reshape(E, HT, P, IT, P).transpose(0, 3, 2, 1, 4).reshape(E, IT, P, H)
    wgu_all = np.ascontiguousarray(np.stack([wg_slab, wu_slab], axis=2))  # [E, IT, 2, P, H]
    wd_rows = np.ascontiguousarray(wd_bf.reshape(E, IT, P, H))
    _weight_cache.clear()
    _weight_cache[key] = (wgu_all, wd_rows)
    return _weight_cache[key]


def _route(X, WG):
    """f64 replica of the reference router; returns per-expert dispatch."""
    logits = X.astype(np.float64) @ np.asarray(WG, np.float64)
    scores = 1.0 / (1.0 + np.exp(-logits))
    top2 = np.argsort(-scores, axis=1, kind="stable")[:, :K]
    w = np.take_along_axis(scores, top2, 1)
    wn = (w / w.sum(1, keepdims=True)).astype(np.float32)
    tok_list, w_list = [], []
    for e in range(E):
        hit = top2 == e  # [T, K]
        tok = np.nonzero(hit.any(1))[0]
        kk = hit[tok, 1].astype(np.int64)
        tok_list.append(tok)
        w_list.append(wn[tok, kk])
    return tok_list, w_list


def _pack_xt(X, tokens, cap):
    """[P, HT, cap] bf16: xt[p, h, k] = X[tokens[k], h*P + p]."""
    n = len(tokens)
    Xg = np.zeros((cap, H), np.float32)
    Xg[:n] = X[tokens]
    arr = Xg.T.reshape(HT, P, cap).transpose(1, 0, 2)
    return np.ascontiguousarray(arr.astype(nbf))


def _run(inputs: dict, trace: bool = False, trace_cores=None):
    X = np.ascontiguousarray(np.asarray(inputs["hidden_states"], np.float32))
    tok_list, w_list = _route(X, inputs["w_gate"])
    counts = np.array([len(t) for t in tok_list])

    # slot assignment: heaviest 8 experts in slot 0, lightest 8 in slot 1
    order = np.argsort(-counts, kind="stable")
    slot_exp = [order[:NCORES], order[NCORES:]]  # [slot][core] -> expert
    caps = tuple(
        min(512, max(16, int(counts[slot_exp[j]].max())))
        for j in range(EPC)
    )
    if counts.max() > 512:
        raise RuntimeError(f"expert load {counts.max()} exceeds supported capacity")
    nc = _get_kernel(caps)
    wgu_all, wd_rows = _prep_weights(
        inputs["w_gate_proj"], inputs["w_up_proj"], inputs["w_down_proj"]
    )

    in_maps = []
    for c in range(NCORES):
        experts = [int(slot_exp[j][c]) for j in range(EPC)]
        m = {
            "wgu": np.ascontiguousarray(wgu_all[experts]),
            "wds": np.ascontiguousarray(wd_rows[experts]),
        }
        for j, e in enumerate(experts):
            m[f"xt{j}"] = _pack_xt(X, tok_list[e], caps[j])
        in_maps.append(m)

    if trace:
        _install_trace_shim()
    res = run_bass_kernel_spmd(
        nc,
        in_maps,
        core_ids=list(range(NCORES)),
        trace=trace,
        trace_cores=trace_cores,
    )

    # combine on host: out[t] = w0*Y[:, col0] + w1*Y[:, col1]
    # big: [H, NCORES*(C0+C1)] in (core, slot) column order
    col_blocks = []
    for c in range(NCORES):
        for j in range(EPC):
            y = np.asarray(res.results[c][f"yo{j}"], np.float32)  # [P, HT, Cj]
            col_blocks.append(y.transpose(1, 0, 2).reshape(H, caps[j]))
    big = np.concatenate(col_blocks, axis=1)
    TCc = sum(caps)

    col_a = np.full(T, -1, np.int64)
    col_b = np.full(T, -1, np.int64)
    w_a = np.zeros(T, np.float32)
    w_b = np.zeros(T, np.float32)
    for j in range(EPC):
        base_j = sum(caps[:j])
        for c in range(NCORES):
            e = int(slot_exp[j][c])
            tok = tok_list[e]
            cols = c * TCc + base_j + np.arange(counts[e])
            first = col_a[tok] < 0
            col_a[tok[first]] = cols[first]
            w_a[tok[first]] = w_list[e][first]
            col_b[tok[~first]] = cols[~first]
            w_b[tok[~first]] = w_list[e][~first]
    assert (col_a >= 0).all() and (col_b >= 0).all()
    out = (big[:, col_a] * w_a[None, :] + big[:, col_b] * w_b[None, :]).T
    return np.ascontiguousarray(out.astype(np.float32)), res


def kernel(**inputs) -> np.ndarray:
    out, _ = _run(inputs, trace=False)
    return out


def _install_trace_shim():
    """Make run_bass_kernel_spmd(trace=True) work under axon: register the
    NTFF profile hook that the slim agent image's antenv stub lacks."""
    import sys, types

    if "antenv.axon_hooks" not in sys.modules:
        import antenv

        mod = types.ModuleType("antenv.axon_hooks")
        mod._hook = None
        mod.set_axon_ntff_profile_hook = lambda h: setattr(mod, "_hook", h)
        mod.get_axon_ntff_profile_hook = lambda: mod._hook
        sys.modules["antenv.axon_hooks"] = mod
        antenv.axon_hooks = mod
    if sys.modules["antenv.axon_hooks"].get_axon_ntff_profile_hook() is None:
        from trn_agent_boot.trn_boot import _ntff_profile_via_ctypes

        sys.modules["antenv.axon_hooks"].set_axon_ntff_profile_hook(
            _ntff_profile_via_ctypes("/opt/axon/libaxon_pjrt.so")
        )
